# revision 23
# baseline (speedup 1.0000x reference)
"""Multi-head attention (16 heads, d=64, d_model=1024, SL=2048, BS=2) on 8
Trainium2 NeuronCores.

Sharding: core c handles batch b = c // 4 and heads [4*(c%4), 4*(c%4)+4).
Each core computes a partial output y_c[2048, 1024] (its 4 heads' contribution
through Wo for its batch); the host sums the 4 partials per batch.

Host-side prep feeds activations TRANSPOSED ([d_model, seq]) so every on-chip
matmul has its contraction dim on partitions; no on-chip transposes needed.

Performance notes (v2):
- All PE moving operands are BF16: fp32/f32r moving streams at ~half rate
  (427 ns vs 213 ns for N=512), measured via NTFF trace of v1.
- Q^T/K^T/V/A^T/p all stored BF16; Wo streamed BF16. The scores matmuls are
  row-tiled (K=64 head dim -> tile_position (0,0)/(64,0) auto-derived from
  base partitions) so head pairs run concurrently in the PE array halves.
- exp is split across two engines: ACT does head hl=0 exactly, DVE does
  head hl=1 with a one-instruction Schraudolph: p_bf16bits =
  int16(s*128/ln2 + (127*128 + delta)), bitcast to bf16. Softmax
  normalization cancels the constant bias; residual rel err ~3% on half the
  heads, which washes out through softmax + Wo averaging (measured 1.5e-2
  total vs the 2e-2 gate).
- Projections run chunk-inner (each arriving d_model chunk feeds all its
  matmuls immediately) so DMA pipelines with PE and the HAM clock stays warm.
- per (q-chunk 512, head): S^T[k,q] tiles -> exp -> attnU^T[65,512]
  accumulated over 16 k-tiles; row 64 = softmax denominator l (ones column
  in V). normalize: recip(l) -> gpsimd broadcast -> DVE multiply -> A^T bf16.
  O-proj: stationary A^T chunk, moving Wo^T bf16, y bf16 out.
"""

import os
import sys
for _p in ("/opt/trn_rl_repo", "/root/.axon_site/_ro/trn_rl_repo"):
    if os.path.isdir(_p) and _p not in sys.path:
        sys.path.insert(0, _p)

import numpy as np

import concourse.bass as bass
import concourse.tile as tile
from concourse import bacc, mybir
from concourse.bass_utils import run_bass_kernel_spmd

N_CORES = 8
SL = 2048
BS = 2
DM = 1024          # d_model
H = 16             # total heads
DH = 64            # head dim
HPC = 4            # heads per core
IC = HPC * DH      # per-core inner dim = 256
F32 = mybir.dt.float32
BF16 = mybir.dt.bfloat16
I16 = mybir.dt.int16
F32R = mybir.dt.float32r
Exp = mybir.ActivationFunctionType.Exp
MUL = mybir.AluOpType.mult
ADD = mybir.AluOpType.add

N_DMC = DM // 128          # 8 d_model chunks
N_KT = SL // 128           # 16 k tiles
N_QC = SL // 512           # 4 q chunks
VW = 65                    # V columns per head incl. ones column
VBLK = HPC * VW            # 260 V columns per k-tile block

# Schraudolph exp in bf16-bit domain: int16(s*C1 + C2) viewed as bf16.
EXP_C1 = float(np.float32(128.0 / np.log(2.0)))
EXP_C2 = float(np.float32(127.0 * 128.0 - 5.25))


def build_kernel(reps=1):
    nc = bacc.Bacc("TRN2", target_bir_lowering=False, debug=False,
                   num_devices=N_CORES)
    qT = nc.dram_tensor("qT", [DM, SL], BF16, kind="ExternalInput").ap()
    kT = nc.dram_tensor("kT", [DM, SL], BF16, kind="ExternalInput").ap()
    vT = nc.dram_tensor("vT", [DM, SL], BF16, kind="ExternalInput").ap()
    wqT = nc.dram_tensor("wqT", [DM, IC], BF16, kind="ExternalInput").ap()
    wkT = nc.dram_tensor("wkT", [DM, IC], BF16, kind="ExternalInput").ap()
    wvT = nc.dram_tensor("wvT", [DM, IC], BF16, kind="ExternalInput").ap()
    woT = nc.dram_tensor("woT", [IC, DM], BF16, kind="ExternalInput").ap()
    Y = nc.dram_tensor("Y", [SL, DM], BF16, kind="ExternalOutput").ap()

    with tile.TileContext(nc) as tc:
        _build_body(nc, tc, qT, kT, vT, wqT, wkT, wvT, woT, Y, reps)
    nc.compile()
    return nc


def _build_body(nc, tc, qT, kT, vT, wqT, wkT, wvT, woT, Y, reps=1):
    import contextlib
    ctx = contextlib.ExitStack()
    with ctx:
        wpool = ctx.enter_context(tc.tile_pool(name="w", bufs=1))
        xin = ctx.enter_context(tc.tile_pool(name="xin", bufs=16))
        qk = ctx.enter_context(tc.tile_pool(name="qk", bufs=1))
        vpool = ctx.enter_context(tc.tile_pool(name="v", bufs=1))
        ptp = ctx.enter_context(tc.tile_pool(name="pt", bufs=6))
        atp = ctx.enter_context(tc.tile_pool(name="at", bufs=1))
        ypool = ctx.enter_context(tc.tile_pool(name="y", bufs=2))
        misc = ctx.enter_context(tc.tile_pool(name="misc", bufs=2))
        # PSUM: ps 2x[128,1024] (4 banks) + psu 3x[128,512] + psy 1x[128,512]
        # 6 + 2 = 8 PSUM banks (o-proj yp shares the "s" tag slots)
        ps = ctx.enter_context(tc.tile_pool(name="ps", bufs=6, space="PSUM"))
        psu = ctx.enter_context(tc.tile_pool(name="psu", bufs=2, space="PSUM"))

        # ---- weights (tiles allocated up-front; DMAs issued near first use)
        w_sb = {}
        w_dram = {"wq": wqT, "wk": wkT, "wv": wvT}
        for name in ("wq", "wk", "wv"):
            w_sb[name] = wpool.tile([128, N_DMC * IC], BF16, tag=name,
                                    name=name)

        def load_w(name):
            wT = w_dram[name]
            t = w_sb[name]
            nc.scalar.dma_start(
                out=t[:].rearrange("p (c f) -> p c f", c=N_DMC),
                in_=wT.rearrange("(c p) f -> p c f", p=128))

        wo_sb = []
        for i in range(2):
            t = wpool.tile([128, DM], BF16, tag=f"wo{i}", name=f"wo{i}")
            wo_sb.append(t)

        def load_wo():
            for i in range(2):
                nc.scalar.dma_start(out=wo_sb[i][:],
                                    in_=woT[i * 128:(i + 1) * 128, :])

        ones_f32 = misc.tile([128, DH], F32, tag="ones_f32")
        nc.vector.memset(ones_f32[:], 1.0)
        exp_warm = misc.tile([1, 8], F32, tag="exp_warm")

        # ---- long-lived activations ----
        QT = [qk.tile([128, SL], BF16, tag=f"qt{p}", name=f"qt{p}")
              for p in range(2)]
        KT = [qk.tile([128, SL], BF16, tag=f"kt{p}", name=f"kt{p}")
              for p in range(2)]
        AT = [atp.tile([128, SL], BF16, tag=f"at{p}", name=f"at{p}")
              for p in range(2)]
        V = vpool.tile([128, N_KT * VBLK], BF16, tag="vsb")

        def copy_psum(use_scalar, out, in_):
            if use_scalar:
                nc.scalar.copy(out=out, in_=in_)
            else:
                nc.vector.tensor_copy(out, in_)

        def proj_qk(xdram, wname, out_tiles, scalar_first):
            # chunk-inner: each x chunk feeds 4 of the 8 (hp, tcq) psum
            # accumulators; two sub-phases of 4 accumulators (2 ps tiles).
            chunks = []
            for c in range(N_DMC):
                xt = xin.tile([128, SL], BF16, tag="xin")
                nc.sync.dma_start(out=xt[:], in_=xdram[c * 128:(c + 1) * 128, :])
                chunks.append(xt)
            for sub in range(2):        # hp = sub, all 4 tcq accs live at once
                accs = [ps.tile([128, 512], F32, tag="s", name=f"acc{_j}")
                        for _j in range(4)]
                for c in range(N_DMC):
                    for tcq in range(4):
                        nc.tensor.matmul(
                            accs[tcq][:],
                            w_sb[wname][:, c * IC + sub * 128:
                                        c * IC + (sub + 1) * 128],
                            chunks[c][:, tcq * 512:(tcq + 1) * 512],
                            start=(c == 0), stop=(c == N_DMC - 1))
                for tcq in range(4):
                    copy_psum((tcq % 2 == 0) == scalar_first,
                              out_tiles[sub][:, tcq * 512:(tcq + 1) * 512],
                              accs[tcq][:])

        def proj_v():
            chunks = []
            for c in range(N_DMC):
                xt = xin.tile([128, SL], BF16, tag="xin")
                nc.sync.dma_start(out=xt[:], in_=vT[c * 128:(c + 1) * 128, :])
                chunks.append(xt)
            # ones columns of V (col 64 of each head's 65-wide block)
            for h in range(HPC):
                nc.vector.tensor_copy(V[:, h * VW + 64::VBLK],
                                      ones_f32[:, 0:N_KT])
            for kt in range(N_KT):
                acc = psu.tile([128, 512], F32, tag="accu")
                for c in range(N_DMC):
                    nc.tensor.matmul(
                        acc[:, 0:IC],
                        chunks[c][:, kt * 128:(kt + 1) * 128],
                        w_sb["wv"][:, c * IC:(c + 1) * IC],
                        start=(c == 0), stop=(c == N_DMC - 1))
                copy_psum(kt % 2 == 0,
                          V[:, kt * VBLK:(kt + 1) * VBLK].rearrange(
                              "p (h d) -> p h d", d=VW)[:, :, 0:DH],
                          acc[:, 0:IC].rearrange("p (h d) -> p h d", h=HPC))

        for _rep in range(reps):
            load_w("wk")
            load_w("wq")
            load_w("wv")
            load_wo()
            # warm the ACT exp table set during the projection phase
            nc.scalar.activation(exp_warm[:], ones_f32[0:1, 0:8], Exp)

            proj_qk(kT, "wk", KT, False)
            proj_qk(qT, "wq", QT, True)
            proj_v()

            # ---- attention + o-proj, per q-chunk ----
            def emit_scores(qc, pair, kt):
                pts = []
                for hl in range(2):
                    s = ps.tile([128, 512], F32, tag="s", name="s")
                    nc.tensor.matmul(
                        s[:],
                        KT[pair][hl * 64:(hl + 1) * 64,
                                 kt * 128:(kt + 1) * 128],
                        QT[pair][hl * 64:(hl + 1) * 64,
                                 qc * 512:(qc + 1) * 512],
                        start=True, stop=True)
                    p = ptp.tile([128, 512], BF16, tag="pt", name="p")
                    if hl == 0:
                        nc.scalar.activation(p[:], s[:], Exp)
                    else:
                        nc.vector.tensor_scalar(
                            p[:].bitcast(I16), s[:], EXP_C1, EXP_C2, MUL, ADD)
                    pts.append(p)
                return pts

            def emit_av(au, pair, kt, pts):
                for hl in range(2):
                    h = pair * 2 + hl
                    nc.tensor.matmul(
                        au[hl][0:VW, :],
                        V[:, kt * VBLK + h * VW:kt * VBLK + (h + 1) * VW],
                        pts[hl][:],
                        start=(kt == 0), stop=(kt == N_KT - 1))

            def emit_oproj_qt(qt):
                y_sb = ypool.tile([128, DM], BF16, tag="ysb", name="y_sb")
                for mh in range(2):            # output halves of 1024
                    yp = ps.tile([128, 512], F32, tag="s", name="yp")
                    for ich in range(2):       # i chunks (AT0, AT1)
                        nc.tensor.matmul(
                            yp[:],
                            AT[ich][:, qt * 128:(qt + 1) * 128],
                            wo_sb[ich][:, mh * 512:(mh + 1) * 512],
                            start=(ich == 0), stop=(ich == 1))
                    copy_psum(True, y_sb[:, mh * 512:(mh + 1) * 512], yp[:])
                nc.gpsimd.dma_start(out=Y[qt * 128:(qt + 1) * 128, :],
                                    in_=y_sb[:])

            def emit_evac(au):
                # Evacuate au with two fast parallel copies so the au banks
                # free after ~700 ns (the next pair's AV allocation waits on
                # them); the normalize tail runs later from SBUF via the drip.
                st = []
                for hl in range(2):
                    araw = misc.tile([64, 512], BF16, tag="araw", name="araw")
                    nc.scalar.copy(out=araw[:], in_=au[hl][0:64, :])
                    l_sb = misc.tile([1, 512], F32, tag="l_sb", name="l_sb")
                    nc.vector.tensor_copy(l_sb[:], au[hl][64:65, :])
                    st.append((araw, l_sb))
                return st

            def norm_closures(qc, pair, st):
                # recip -> broadcast -> mul, all from SBUF, dripped so none of
                # it sits ahead of the next pair's exp work in engine FIFOs.
                out = []
                for hl in range(2):
                    araw, l_sb = st[hl]
                    ctx2 = {}

                    def recip(ctx2=ctx2, l_sb=l_sb):
                        ctx2["rc"] = misc.tile([1, 512], F32, tag="rc",
                                               name="rc")
                        nc.vector.reciprocal_approx_fast(out=ctx2["rc"][:],
                                                         in_=l_sb[:])
                        ctx2["rb"] = misc.tile([64, 512], F32, tag="rb",
                                               name="rb")
                        nc.gpsimd.partition_broadcast(ctx2["rb"][:],
                                                      ctx2["rc"][:])

                    def mul(hl=hl, ctx2=ctx2, araw=araw):
                        # gpsimd: slower than DVE but off the exp path, and
                        # its rb dependency is same-queue (no head-blocking)
                        nc.gpsimd.tensor_mul(
                            AT[pair][hl * 64:(hl + 1) * 64,
                                     qc * 512:(qc + 1) * 512],
                            araw[:], ctx2["rb"][:])

                    out += [recip, mul]
                return out

            drip = []               # deferred closures, one emitted per kt
            for qc in range(N_QC):
                for pair in range(2):
                    au = [psu.tile([128, 512], F32, tag="accu", name=f"au{hl}")
                          for hl in range(2)]
                    # software pipeline: scores run TWO kt ahead of AV (6
                    # s-banks) so exp(kt) completes behind two slots of queued
                    # PE work and AV never waits; deferred normalize/o-proj
                    # closures drip one per kt.
                    pts0 = emit_scores(qc, pair, 0)
                    pts1 = emit_scores(qc, pair, 1)
                    for kt in range(N_KT):
                        nxt = emit_scores(qc, pair, kt + 2) \
                            if kt + 2 < N_KT else None
                        emit_av(au, pair, kt, pts0)
                        pts0, pts1 = pts1, nxt
                        if kt >= 1 and drip:
                            drip.pop(0)()
                    st = emit_evac(au)
                    drip.extend(norm_closures(qc, pair, st))
                drip.extend((lambda qt=qt: emit_oproj_qt(qt))
                            for qt in range(4 * qc, 4 * (qc + 1)))
            for c in drip:
                c()


_NC_CACHE = None


def _get_nc():
    global _NC_CACHE
    if _NC_CACHE is None:
        _NC_CACHE = build_kernel()
    return _NC_CACHE


def make_in_maps(query, keys, values, Wq, Wk, Wv, Wo):
    query = np.ascontiguousarray(query, dtype=np.float32)
    keys = np.ascontiguousarray(keys, dtype=np.float32)
    values = np.ascontiguousarray(values, dtype=np.float32)
    import ml_dtypes
    bf16 = ml_dtypes.bfloat16
    xTs = {}
    for b in range(BS):
        xTs[b] = (
            np.ascontiguousarray(query[:, b, :].T.astype(bf16)),
            np.ascontiguousarray(keys[:, b, :].T.astype(bf16)),
            np.ascontiguousarray(values[:, b, :].T.astype(bf16)),
        )
    wTs = {}
    for g in range(N_CORES // BS):
        sl = slice(g * IC, (g + 1) * IC)
        wTs[g] = (
            np.ascontiguousarray(np.asarray(Wq, dtype=np.float32)[sl, :].T.astype(bf16)),
            np.ascontiguousarray(np.asarray(Wk, dtype=np.float32)[sl, :].T.astype(bf16)),
            np.ascontiguousarray(np.asarray(Wv, dtype=np.float32)[sl, :].T.astype(bf16)),
            np.ascontiguousarray(np.asarray(Wo, dtype=np.float32)[:, sl].T.astype(bf16)),
        )
    in_maps = []
    for c in range(N_CORES):
        b, g = c // 4, c % 4
        qTb, kTb, vTb = xTs[b]
        wq, wk, wv, wo = wTs[g]
        in_maps.append({"qT": qTb, "kT": kTb, "vT": vTb,
                        "wqT": wq, "wkT": wk, "wvT": wv, "woT": wo})
    return in_maps


def assemble_output(results):
    out = np.zeros((SL, BS, DM), dtype=np.float32)
    for c in range(N_CORES):
        b = c // 4
        out[:, b, :] += np.asarray(results[c]["Y"], dtype=np.float32)
    return out


def kernel(query, keys, values, Wq, Wk, Wv, Wo):
    nc = _get_nc()
    in_maps = make_in_maps(query, keys, values, Wq, Wk, Wv, Wo)
    res = run_bass_kernel_spmd(nc, in_maps, list(range(N_CORES)))
    return assemble_output(res.results)


# revision 24
# speedup vs baseline: 1.8538x; 1.8538x over previous
"""Multi-head attention (16 heads, d=64, d_model=1024, SL=2048, BS=2) on 8
Trainium2 NeuronCores.

Sharding: core c handles batch b = c // 4 and heads [4*(c%4), 4*(c%4)+4).
Each core computes a partial output y_c[2048, 1024] (its 4 heads' contribution
through Wo for its batch); the host sums the 4 partials per batch.

Host-side prep feeds activations TRANSPOSED ([d_model, seq]) so every on-chip
matmul has its contraction dim on partitions; no on-chip transposes needed.

Performance notes (v2):
- All PE moving operands are BF16: fp32/f32r moving streams at ~half rate
  (427 ns vs 213 ns for N=512), measured via NTFF trace of v1.
- Q^T/K^T/V/A^T/p all stored BF16; Wo streamed BF16. The scores matmuls are
  row-tiled (K=64 head dim -> tile_position (0,0)/(64,0) auto-derived from
  base partitions) so head pairs run concurrently in the PE array halves.
- exp is split across two engines: ACT does head hl=0 exactly, DVE does
  head hl=1 with a one-instruction Schraudolph: p_bf16bits =
  int16(s*128/ln2 + (127*128 + delta)), bitcast to bf16. Softmax
  normalization cancels the constant bias; residual rel err ~3% on half the
  heads, which washes out through softmax + Wo averaging (measured 1.5e-2
  total vs the 2e-2 gate).
- Projections run chunk-inner (each arriving d_model chunk feeds all its
  matmuls immediately) so DMA pipelines with PE and the HAM clock stays warm.
- per (q-chunk 512, head): S^T[k,q] tiles -> exp -> attnU^T[65,512]
  accumulated over 16 k-tiles; row 64 = softmax denominator l (ones column
  in V). normalize: recip(l) -> gpsimd broadcast -> DVE multiply -> A^T bf16.
  O-proj: stationary A^T chunk, moving Wo^T bf16, y bf16 out.
"""

import os
import sys
for _p in ("/opt/trn_rl_repo", "/root/.axon_site/_ro/trn_rl_repo"):
    if os.path.isdir(_p) and _p not in sys.path:
        sys.path.insert(0, _p)

import numpy as np

import concourse.bass as bass
import concourse.tile as tile
from concourse import bacc, mybir
from concourse.bass_utils import run_bass_kernel_spmd

N_CORES = 8
SL = 2048
BS = 2
DM = 1024          # d_model
H = 16             # total heads
DH = 64            # head dim
HPC = 4            # heads per core
IC = HPC * DH      # per-core inner dim = 256
F32 = mybir.dt.float32
BF16 = mybir.dt.bfloat16
I16 = mybir.dt.int16
F32R = mybir.dt.float32r
Exp = mybir.ActivationFunctionType.Exp
MUL = mybir.AluOpType.mult
ADD = mybir.AluOpType.add

N_DMC = DM // 128          # 8 d_model chunks
N_KT = SL // 128           # 16 k tiles
N_QC = SL // 512           # 4 q chunks
VW = 65                    # V columns per head incl. ones column
VBLK = HPC * VW            # 260 V columns per k-tile block

# Schraudolph exp in bf16-bit domain: int16(s*C1 + C2) viewed as bf16.
EXP_C1 = float(np.float32(128.0 / np.log(2.0)))
EXP_C2 = float(np.float32(127.0 * 128.0 - 5.25))


def build_kernel(reps=1):
    nc = bacc.Bacc("TRN2", target_bir_lowering=False, debug=False,
                   num_devices=N_CORES)
    qT = nc.dram_tensor("qT", [DM, SL], BF16, kind="ExternalInput").ap()
    kT = nc.dram_tensor("kT", [DM, SL], BF16, kind="ExternalInput").ap()
    vT = nc.dram_tensor("vT", [DM, SL], BF16, kind="ExternalInput").ap()
    wqT = nc.dram_tensor("wqT", [DM, IC], BF16, kind="ExternalInput").ap()
    wkT = nc.dram_tensor("wkT", [DM, IC], BF16, kind="ExternalInput").ap()
    wvT = nc.dram_tensor("wvT", [DM, IC], BF16, kind="ExternalInput").ap()
    woT = nc.dram_tensor("woT", [IC, DM], BF16, kind="ExternalInput").ap()
    Y = nc.dram_tensor("Y", [SL, DM], BF16, kind="ExternalOutput").ap()

    with tile.TileContext(nc) as tc:
        _build_body(nc, tc, qT, kT, vT, wqT, wkT, wvT, woT, Y, reps)
    nc.compile()
    return nc


def _build_body(nc, tc, qT, kT, vT, wqT, wkT, wvT, woT, Y, reps=1):
    import contextlib
    ctx = contextlib.ExitStack()
    with ctx:
        wpool = ctx.enter_context(tc.tile_pool(name="w", bufs=1))
        xin = ctx.enter_context(tc.tile_pool(name="xin", bufs=16))
        qk = ctx.enter_context(tc.tile_pool(name="qk", bufs=1))
        vpool = ctx.enter_context(tc.tile_pool(name="v", bufs=1))
        ptp = ctx.enter_context(tc.tile_pool(name="pt", bufs=6))
        atp = ctx.enter_context(tc.tile_pool(name="at", bufs=1))
        ypool = ctx.enter_context(tc.tile_pool(name="y", bufs=2))
        misc = ctx.enter_context(tc.tile_pool(name="misc", bufs=2))
        # PSUM: ps 2x[128,1024] (4 banks) + psu 3x[128,512] + psy 1x[128,512]
        # 6 + 2 = 8 PSUM banks (o-proj yp shares the "s" tag slots)
        ps = ctx.enter_context(tc.tile_pool(name="ps", bufs=6, space="PSUM"))
        psu = ctx.enter_context(tc.tile_pool(name="psu", bufs=2, space="PSUM"))

        # ---- weights (tiles allocated up-front; DMAs issued near first use)
        w_sb = {}
        w_dram = {"wq": wqT, "wk": wkT, "wv": wvT}
        for name in ("wq", "wk", "wv"):
            w_sb[name] = wpool.tile([128, N_DMC * IC], BF16, tag=name,
                                    name=name)

        def load_w(name):
            wT = w_dram[name]
            t = w_sb[name]
            nc.scalar.dma_start(
                out=t[:].rearrange("p (c f) -> p c f", c=N_DMC),
                in_=wT.rearrange("(c p) f -> p c f", p=128))

        wo_sb = []
        for i in range(2):
            t = wpool.tile([128, DM], BF16, tag=f"wo{i}", name=f"wo{i}")
            wo_sb.append(t)

        def load_wo():
            for i in range(2):
                nc.scalar.dma_start(out=wo_sb[i][:],
                                    in_=woT[i * 128:(i + 1) * 128, :])

        ones_f32 = misc.tile([128, DH], F32, tag="ones_f32")
        nc.vector.memset(ones_f32[:], 1.0)
        exp_warm = misc.tile([1, 8], F32, tag="exp_warm")

        # ---- long-lived activations ----
        QT = [qk.tile([128, SL], BF16, tag=f"qt{p}", name=f"qt{p}")
              for p in range(2)]
        KT = [qk.tile([128, SL], BF16, tag=f"kt{p}", name=f"kt{p}")
              for p in range(2)]
        AT = [atp.tile([128, SL], BF16, tag=f"at{p}", name=f"at{p}")
              for p in range(2)]
        V = vpool.tile([128, N_KT * VBLK], BF16, tag="vsb")

        def copy_psum(use_scalar, out, in_):
            if use_scalar:
                nc.scalar.copy(out=out, in_=in_)
            else:
                nc.vector.tensor_copy(out, in_)

        def proj_qk(xdram, wname, out_tiles, scalar_first):
            # chunk-inner: each x chunk feeds 4 of the 8 (hp, tcq) psum
            # accumulators; two sub-phases of 4 accumulators (2 ps tiles).
            chunks = []
            for c in range(N_DMC):
                xt = xin.tile([128, SL], BF16, tag="xin")
                nc.sync.dma_start(out=xt[:], in_=xdram[c * 128:(c + 1) * 128, :])
                chunks.append(xt)
            for sub in range(2):        # hp = sub, all 4 tcq accs live at once
                accs = [ps.tile([128, 512], F32, tag="s", name=f"acc{_j}")
                        for _j in range(4)]
                for c in range(N_DMC):
                    for tcq in range(4):
                        nc.tensor.matmul(
                            accs[tcq][:],
                            w_sb[wname][:, c * IC + sub * 128:
                                        c * IC + (sub + 1) * 128],
                            chunks[c][:, tcq * 512:(tcq + 1) * 512],
                            start=(c == 0), stop=(c == N_DMC - 1))
                for tcq in range(4):
                    copy_psum((tcq % 2 == 0) == scalar_first,
                              out_tiles[sub][:, tcq * 512:(tcq + 1) * 512],
                              accs[tcq][:])

        def proj_v():
            chunks = []
            for c in range(N_DMC):
                xt = xin.tile([128, SL], BF16, tag="xin")
                nc.sync.dma_start(out=xt[:], in_=vT[c * 128:(c + 1) * 128, :])
                chunks.append(xt)
            # ones columns of V (col 64 of each head's 65-wide block)
            for h in range(HPC):
                nc.vector.tensor_copy(V[:, h * VW + 64::VBLK],
                                      ones_f32[:, 0:N_KT])
            for kt in range(N_KT):
                acc = psu.tile([128, 512], F32, tag="accu")
                for c in range(N_DMC):
                    nc.tensor.matmul(
                        acc[:, 0:IC],
                        chunks[c][:, kt * 128:(kt + 1) * 128],
                        w_sb["wv"][:, c * IC:(c + 1) * IC],
                        start=(c == 0), stop=(c == N_DMC - 1))
                copy_psum(kt % 2 == 0,
                          V[:, kt * VBLK:(kt + 1) * VBLK].rearrange(
                              "p (h d) -> p h d", d=VW)[:, :, 0:DH],
                          acc[:, 0:IC].rearrange("p (h d) -> p h d", h=HPC))

        for _rep in range(reps):
            load_w("wk")
            load_w("wq")
            load_w("wv")
            load_wo()
            # warm the ACT exp table set during the projection phase
            nc.scalar.activation(exp_warm[:], ones_f32[0:1, 0:8], Exp)

            proj_qk(kT, "wk", KT, False)
            proj_qk(qT, "wq", QT, True)
            proj_v()

            # ---- attention + o-proj, per q-chunk ----
            def emit_scores(qc, pair, kt):
                pts = []
                for hl in range(2):
                    s = ps.tile([128, 512], F32, tag="s", name="s")
                    nc.tensor.matmul(
                        s[:],
                        KT[pair][hl * 64:(hl + 1) * 64,
                                 kt * 128:(kt + 1) * 128],
                        QT[pair][hl * 64:(hl + 1) * 64,
                                 qc * 512:(qc + 1) * 512],
                        start=True, stop=True)
                    p = ptp.tile([128, 512], BF16, tag="pt", name="p")
                    if hl == 0:
                        nc.scalar.activation(p[:], s[:], Exp)
                    else:
                        nc.vector.tensor_scalar(
                            p[:].bitcast(I16), s[:], EXP_C1, EXP_C2, MUL, ADD)
                    pts.append(p)
                return pts

            def emit_av(au, pair, kt, pts):
                for hl in range(2):
                    h = pair * 2 + hl
                    nc.tensor.matmul(
                        au[hl][0:VW, :],
                        V[:, kt * VBLK + h * VW:kt * VBLK + (h + 1) * VW],
                        pts[hl][:],
                        start=(kt == 0), stop=(kt == N_KT - 1))

            def emit_oproj_qt(qt):
                y_sb = ypool.tile([128, DM], BF16, tag="ysb", name="y_sb")
                for mh in range(2):            # output halves of 1024
                    yp = ps.tile([128, 512], F32, tag="s", name="yp")
                    for ich in range(2):       # i chunks (AT0, AT1)
                        nc.tensor.matmul(
                            yp[:],
                            AT[ich][:, qt * 128:(qt + 1) * 128],
                            wo_sb[ich][:, mh * 512:(mh + 1) * 512],
                            start=(ich == 0), stop=(ich == 1))
                    copy_psum(True, y_sb[:, mh * 512:(mh + 1) * 512], yp[:])
                nc.gpsimd.dma_start(out=Y[qt * 128:(qt + 1) * 128, :],
                                    in_=y_sb[:])

            def emit_evac(au):
                # Evacuate au with two fast parallel copies so the au banks
                # free after ~700 ns (the next pair's AV allocation waits on
                # them); the normalize tail runs later from SBUF via the drip.
                st = []
                for hl in range(2):
                    araw = misc.tile([64, 512], BF16, tag="araw", name="araw")
                    nc.scalar.copy(out=araw[:], in_=au[hl][0:64, :])
                    l_sb = misc.tile([1, 512], F32, tag="l_sb", name="l_sb")
                    nc.vector.tensor_copy(l_sb[:], au[hl][64:65, :])
                    st.append((araw, l_sb))
                return st

            def norm_closures(qc, pair, st):
                # recip+broadcast then mul, all from SBUF, dripped so none of
                # it sits ahead of the next pair's exp work in engine FIFOs.
                # Order [recip0, recip1, mul0, mul1]: each mul pops ~2 kt
                # after its recip, so the gpsimd broadcast (~1 us) finishes
                # before the mul reaches the DVE queue head (no blocking).
                recips, muls = [], []
                for hl in range(2):
                    araw, l_sb = st[hl]
                    ctx2 = {}

                    def recip(ctx2=ctx2, l_sb=l_sb):
                        ctx2["rc"] = misc.tile([1, 512], F32, tag="rc",
                                               name="rc")
                        nc.vector.reciprocal_approx_fast(out=ctx2["rc"][:],
                                                         in_=l_sb[:])
                        ctx2["rb"] = misc.tile([64, 512], F32, tag="rb",
                                               name="rb")
                        nc.gpsimd.partition_broadcast(ctx2["rb"][:],
                                                      ctx2["rc"][:])

                    def mul(hl=hl, ctx2=ctx2, araw=araw):
                        nc.vector.tensor_mul(
                            AT[pair][hl * 64:(hl + 1) * 64,
                                     qc * 512:(qc + 1) * 512],
                            araw[:], ctx2["rb"][:])

                    recips.append(recip)
                    muls.append(mul)
                return recips + muls

            drip = []               # deferred closures, one emitted per kt
            for qc in range(N_QC):
                for pair in range(2):
                    au = [psu.tile([128, 512], F32, tag="accu", name=f"au{hl}")
                          for hl in range(2)]
                    # software pipeline: scores run TWO kt ahead of AV (6
                    # s-banks) so exp(kt) completes behind two slots of queued
                    # PE work and AV never waits; deferred normalize/o-proj
                    # closures drip one per kt.
                    pts0 = emit_scores(qc, pair, 0)
                    pts1 = emit_scores(qc, pair, 1)
                    for kt in range(N_KT):
                        nxt = emit_scores(qc, pair, kt + 2) \
                            if kt + 2 < N_KT else None
                        emit_av(au, pair, kt, pts0)
                        pts0, pts1 = pts1, nxt
                        if kt >= 1 and drip:
                            drip.pop(0)()
                    st = emit_evac(au)
                    drip.extend(norm_closures(qc, pair, st))
                drip.extend((lambda qt=qt: emit_oproj_qt(qt))
                            for qt in range(4 * qc, 4 * (qc + 1)))
            for c in drip:
                c()


_NC_CACHE = None


def _get_nc():
    global _NC_CACHE
    if _NC_CACHE is None:
        _NC_CACHE = build_kernel()
    return _NC_CACHE


def make_in_maps(query, keys, values, Wq, Wk, Wv, Wo):
    query = np.ascontiguousarray(query, dtype=np.float32)
    keys = np.ascontiguousarray(keys, dtype=np.float32)
    values = np.ascontiguousarray(values, dtype=np.float32)
    import ml_dtypes
    bf16 = ml_dtypes.bfloat16
    xTs = {}
    for b in range(BS):
        xTs[b] = (
            np.ascontiguousarray(query[:, b, :].T.astype(bf16)),
            np.ascontiguousarray(keys[:, b, :].T.astype(bf16)),
            np.ascontiguousarray(values[:, b, :].T.astype(bf16)),
        )
    wTs = {}
    for g in range(N_CORES // BS):
        sl = slice(g * IC, (g + 1) * IC)
        wTs[g] = (
            np.ascontiguousarray(np.asarray(Wq, dtype=np.float32)[sl, :].T.astype(bf16)),
            np.ascontiguousarray(np.asarray(Wk, dtype=np.float32)[sl, :].T.astype(bf16)),
            np.ascontiguousarray(np.asarray(Wv, dtype=np.float32)[sl, :].T.astype(bf16)),
            np.ascontiguousarray(np.asarray(Wo, dtype=np.float32)[:, sl].T.astype(bf16)),
        )
    in_maps = []
    for c in range(N_CORES):
        b, g = c // 4, c % 4
        qTb, kTb, vTb = xTs[b]
        wq, wk, wv, wo = wTs[g]
        in_maps.append({"qT": qTb, "kT": kTb, "vT": vTb,
                        "wqT": wq, "wkT": wk, "wvT": wv, "woT": wo})
    return in_maps


def assemble_output(results):
    out = np.zeros((SL, BS, DM), dtype=np.float32)
    for c in range(N_CORES):
        b = c // 4
        out[:, b, :] += np.asarray(results[c]["Y"], dtype=np.float32)
    return out


def kernel(query, keys, values, Wq, Wk, Wv, Wo):
    nc = _get_nc()
    in_maps = make_in_maps(query, keys, values, Wq, Wk, Wv, Wo)
    res = run_bass_kernel_spmd(nc, in_maps, list(range(N_CORES)))
    return assemble_output(res.results)
